# revision 6
# baseline (speedup 1.0000x reference)
"""GCCF encoder (3-layer LightGCN-style propagation) on 8 TRN2 NeuronCores.

Strategy (dest-partitioned SpMM):
  - Node dim sharded 8 ways: core d owns dest rows [d*18750, (d+1)*18750).
  - Core-local edges sorted by dest row, grouped into 148 windows of 128
    dest rows, padded to a fixed TPW tiles of 128 edges per window.
  - Per edge-tile: indirect-DMA gather of the 128 source rows from an
    HBM-resident full embedding table; a one-hot "staircase" matrix S
    (S[e, r] = val_e * (rowloc_e == r)) generated on-chip by the vector
    engine; PE matmul  psum[128 rows, 64] += S.T @ msgs  accumulates the
    segment sum for the window.
  - Window epilogue: relu (scalar engine) -> accumulate into SBUF-resident
    acc -> write y to HBM; AllGather (8 cores) rebuilds the full table for
    the next layer.  acc/4 is the output.
All data-dependent structure lives in input tensors (gather indices,
rowloc, val); the Bass program itself is identical on all 8 cores.
"""

import math
import numpy as np

N_USERS = 100000
N_ITEMS = 50000
N_NODES = N_USERS + N_ITEMS
EMB = 64
N_LAYERS = 3
N_CORES = 8
P = 128

ROWS_PER_CORE = N_NODES // N_CORES          # 18750
W_PER_CORE = math.ceil(ROWS_PER_CORE / P)   # 147 -> pad rows to 148*128
ROWS_PAD = W_PER_CORE * P                   # 18816? (147*128=18816)
N_PAD = ROWS_PAD * N_CORES

_COMPILED = {}


def build_program(tpw, n_layers=N_LAYERS, w_per_core=None, rows_pad=None,
                  n_pad=None, msg_bufs=16, s_bufs=2, psum_bufs=4):
    """Build + compile the SPMD Bass program. Returns (nc, names)."""
    import concourse.bass as bass
    from concourse import bacc, tile, mybir

    if w_per_core is None:
        w_per_core = W_PER_CORE
    if rows_pad is None:
        rows_pad = w_per_core * P
    if n_pad is None:
        n_pad = rows_pad * N_CORES
    slots = w_per_core * tpw

    f32 = mybir.dt.float32
    i32 = mybir.dt.int32

    nc = bacc.Bacc("TRN2", target_bir_lowering=False, debug=False,
                   enable_asserts=False, num_devices=N_CORES)

    x0 = nc.dram_tensor("x0", [n_pad, EMB], f32, kind="ExternalInput")
    ego_slice = nc.dram_tensor("ego_slice", [rows_pad, EMB], f32,
                               kind="ExternalInput")
    idx_in = nc.dram_tensor("idx", [P, slots], i32, kind="ExternalInput")
    rowloc_in = nc.dram_tensor("rowloc", [P, slots], f32, kind="ExternalInput")
    val_in = nc.dram_tensor("val", [P, slots], f32, kind="ExternalInput")
    out = nc.dram_tensor("out", [rows_pad, EMB], f32, kind="ExternalOutput")

    with tile.TileContext(nc) as tc:
        with tc.tile_pool(name="const", bufs=1) as cpool, \
             tc.tile_pool(name="dram", bufs=1, space="DRAM") as dpool, \
             tc.tile_pool(name="msg", bufs=msg_bufs) as mpool, \
             tc.tile_pool(name="sgen", bufs=s_bufs) as spool, \
             tc.tile_pool(name="epi", bufs=4) as epool, \
             tc.tile_pool(name="psum", bufs=psum_bufs, space="PSUM") as ppool:

            # resident tables
            idx_t = cpool.tile([P, slots], i32, name="idx_t")
            nc.sync.dma_start(idx_t[:], idx_in[:])
            rowloc_t = cpool.tile([P, slots], f32, name="rowloc_t")
            nc.sync.dma_start(rowloc_t[:], rowloc_in[:])
            val_t = cpool.tile([P, slots], f32, name="val_t")
            nc.sync.dma_start(val_t[:], val_in[:])

            # acc [p, w*EMB] holds row (w*128+p)
            acc = cpool.tile([P, w_per_core * EMB], f32, name="acc")
            nc.sync.dma_start(
                acc[:].rearrange("p (w d) -> p w d", d=EMB),
                ego_slice[:].rearrange("(w p) d -> p w d", p=P))

            # iota_f[p, t*128 + r] = r  (repeated tpw times), as f32
            # (f32 iota is exact for 0..127)
            iota_f = cpool.tile([P, tpw * P], f32, name="iota_f")
            nc.gpsimd.iota(iota_f[:].rearrange("p (t r) -> p t r", r=P),
                           pattern=[[0, tpw], [1, P]], channel_multiplier=0,
                           allow_small_or_imprecise_dtypes=True)

            # HBM buffers for inter-layer exchange
            y_hbm = [dpool.tile([rows_pad, EMB], f32, name=f"y_hbm{l}")
                     for l in range(n_layers - 1)]
            xg = [dpool.tile([n_pad, EMB], f32, name=f"xg{l}",
                             addr_space="Shared")
                  for l in range(n_layers - 1)]

            for layer in range(n_layers):
                xsrc = x0 if layer == 0 else xg[layer - 1]
                for w in range(w_per_core):
                    base = w * tpw
                    # one-hot staircase for the whole window (2 DVE ops)
                    s_all = spool.tile([P, tpw * P], f32, tag="s_all",
                                       name="s_all")
                    nc.vector.tensor_tensor(
                        out=s_all[:].rearrange("p (t r) -> p t r", r=P),
                        in0=iota_f[:].rearrange("p (t r) -> p t r", r=P),
                        in1=rowloc_t[:, base:base + tpw].to_broadcast(
                            [P, tpw, P]),
                        op=mybir.AluOpType.is_equal)
                    nc.vector.tensor_tensor(
                        out=s_all[:].rearrange("p (t r) -> p t r", r=P),
                        in0=s_all[:].rearrange("p (t r) -> p t r", r=P),
                        in1=val_t[:, base:base + tpw].to_broadcast(
                            [P, tpw, P]),
                        op=mybir.AluOpType.mult)

                    psum = ppool.tile([P, EMB], f32, name="psum")
                    for t in range(tpw):
                        j = base + t
                        g = mpool.tile([P, EMB], f32, tag="g", name="g")
                        nc.gpsimd.indirect_dma_start(
                            out=g[:], out_offset=None, in_=xsrc[:],
                            in_offset=bass.IndirectOffsetOnAxis(
                                ap=idx_t[:, j:j + 1], axis=0))
                        nc.tensor.matmul(
                            out=psum[:],
                            lhsT=s_all[:, t * P:(t + 1) * P],
                            rhs=g[:],
                            start=(t == 0), stop=(t == tpw - 1))

                    ystage = epool.tile([P, EMB], f32, tag="ystage",
                                        name="ystage")
                    nc.scalar.activation(
                        ystage[:], psum[:],
                        mybir.ActivationFunctionType.Relu)
                    nc.vector.tensor_add(
                        acc[:, w * EMB:(w + 1) * EMB],
                        acc[:, w * EMB:(w + 1) * EMB], ystage[:])
                    if layer < n_layers - 1:
                        nc.sync.dma_start(
                            y_hbm[layer][:].rearrange(
                                "(w2 p) d -> p w2 d", p=P)[:, w, :],
                            ystage[:])
                if layer < n_layers - 1:
                    nc.gpsimd.collective_compute(
                        "AllGather", mybir.AluOpType.bypass,
                        replica_groups=[list(range(N_CORES))],
                        ins=[y_hbm[layer][:]], outs=[xg[layer][:]])

            # out = acc / (n_layers + 1), scaled in place
            nc.scalar.mul(acc[:], acc[:], 1.0 / (n_layers + 1))
            nc.sync.dma_start(
                out[:].rearrange("(w p) d -> p w d", p=P),
                acc[:].rearrange("p (w d) -> p w d", d=EMB))

    nc.compile()
    return nc


def preprocess(user_emb, item_emb, adj_val, adj_row, adj_col,
               rows_per_core=None, w_per_core=None):
    """Host-side: shard + sort edges, build per-core index/rowloc/val tables."""
    if rows_per_core is None:
        rows_per_core = ROWS_PER_CORE
    if w_per_core is None:
        w_per_core = W_PER_CORE
    rows_pad = w_per_core * P
    ego = np.ascontiguousarray(
        np.concatenate([np.asarray(user_emb), np.asarray(item_emb)], axis=0),
        dtype=np.float32)
    n_nodes = ego.shape[0]
    n_cores = N_CORES
    n_pad = rows_pad * n_cores

    row = np.asarray(adj_row).astype(np.int64)
    col = np.asarray(adj_col).astype(np.int64)
    val = np.asarray(adj_val).astype(np.float32)

    core_of = row // rows_per_core
    # padded id of each source node
    pcol = ((col // rows_per_core) * rows_pad + (col % rows_per_core)).astype(
        np.int32)

    per_core = []
    tpw_needed = 1
    for d in range(n_cores):
        m = core_of == d
        r_loc = (row[m] - d * rows_per_core).astype(np.int64)
        w = r_loc >> 7
        rl = (r_loc & 127).astype(np.float32)
        order = np.argsort(w, kind="stable")
        w_s = w[order]
        counts = np.bincount(w_s, minlength=w_per_core)
        tpw_needed = max(tpw_needed, int(math.ceil(counts.max() / P)))
        per_core.append((w_s, rl[order], pcol[m][order], val[m][order],
                         counts))

    tpw = tpw_needed
    slots = w_per_core * tpw

    x0 = np.zeros((n_pad, EMB), np.float32)
    x0.reshape(n_cores, rows_pad, EMB)[:, :rows_per_core] = ego.reshape(
        n_cores, rows_per_core, EMB)

    in_maps = []
    for d in range(n_cores):
        w_s, rl_s, pc_s, v_s, counts = per_core[d]
        idx_np = np.zeros((P, slots), np.int32)
        rowloc_np = np.zeros((P, slots), np.float32)
        val_np = np.zeros((P, slots), np.float32)
        # slot within window
        offs = np.concatenate([[0], np.cumsum(counts)])[:-1]
        s_in_w = np.arange(len(w_s)) - offs[w_s]
        t = s_in_w >> 7
        p = s_in_w & 127
        j = w_s * tpw + t
        idx_np[p, j] = pc_s
        rowloc_np[p, j] = rl_s
        val_np[p, j] = v_s
        in_maps.append({
            "x0": x0,
            "ego_slice": np.ascontiguousarray(
                x0.reshape(n_cores, rows_pad, EMB)[d]),
            "idx": idx_np,
            "rowloc": rowloc_np,
            "val": val_np,
        })
    return tpw, in_maps


def kernel(user_emb, item_emb, adj_val, adj_row, adj_col):
    from concourse.bass_utils import run_bass_kernel_spmd

    tpw, in_maps = preprocess(user_emb, item_emb, adj_val, adj_row, adj_col)

    key = (tpw, W_PER_CORE)
    if key not in _COMPILED:
        _COMPILED[key] = build_program(tpw, w_per_core=W_PER_CORE)
    nc = _COMPILED[key]

    res = run_bass_kernel_spmd(nc, in_maps, core_ids=list(range(N_CORES)))
    rows_pad = W_PER_CORE * P
    full = np.concatenate(
        [res.results[d]["out"][:ROWS_PER_CORE] for d in range(N_CORES)],
        axis=0)
    return full[:N_USERS], full[N_USERS:]



# revision 10
# speedup vs baseline: 1.0586x; 1.0586x over previous
"""GCCF encoder (3-layer LightGCN-style propagation) on 8 TRN2 NeuronCores.

Strategy (dest-partitioned SpMM):
  - Node dim sharded 8 ways: core d owns dest rows [d*18750, (d+1)*18750).
  - Core-local edges sorted by dest row, grouped into 148 windows of 128
    dest rows, padded to a fixed TPW tiles of 128 edges per window.
  - Per edge-tile: indirect-DMA gather of the 128 source rows from an
    HBM-resident full embedding table; a one-hot "staircase" matrix S
    (S[e, r] = val_e * (rowloc_e == r)) generated on-chip by the vector
    engine; PE matmul  psum[128 rows, 64] += S.T @ msgs  accumulates the
    segment sum for the window.
  - Window epilogue: relu (scalar engine) -> accumulate into SBUF-resident
    acc -> write y to HBM; AllGather (8 cores) rebuilds the full table for
    the next layer.  acc/4 is the output.
All data-dependent structure lives in input tensors (gather indices,
rowloc, val); the Bass program itself is identical on all 8 cores.
"""

import math
import numpy as np

N_USERS = 100000
N_ITEMS = 50000
N_NODES = N_USERS + N_ITEMS
EMB = 64
N_LAYERS = 3
N_CORES = 8
P = 128

ROWS_PER_CORE = N_NODES // N_CORES          # 18750
W_PER_CORE = math.ceil(ROWS_PER_CORE / P)   # 147 -> pad rows to 148*128
ROWS_PAD = W_PER_CORE * P                   # 18816? (147*128=18816)
N_PAD = ROWS_PAD * N_CORES

_COMPILED = {}


def build_program(tpw, n_layers=N_LAYERS, w_per_core=None, rows_pad=None,
                  n_pad=None, msg_bufs=32, s_bufs=3, psum_bufs=8):
    """Build + compile the SPMD Bass program. Returns (nc, names)."""
    import concourse.bass as bass
    from concourse import bacc, tile, mybir

    if w_per_core is None:
        w_per_core = W_PER_CORE
    if rows_pad is None:
        rows_pad = w_per_core * P
    if n_pad is None:
        n_pad = rows_pad * N_CORES
    slots = w_per_core * tpw

    f32 = mybir.dt.float32
    i32 = mybir.dt.int32

    nc = bacc.Bacc("TRN2", target_bir_lowering=False, debug=False,
                   enable_asserts=False, num_devices=N_CORES)

    x0 = nc.dram_tensor("x0", [n_pad, EMB], f32, kind="ExternalInput")
    ego_slice = nc.dram_tensor("ego_slice", [rows_pad, EMB], f32,
                               kind="ExternalInput")
    idx_in = nc.dram_tensor("idx", [P, slots], i32, kind="ExternalInput")
    rowloc_in = nc.dram_tensor("rowloc", [P, slots], f32, kind="ExternalInput")
    val_in = nc.dram_tensor("val", [P, slots], f32, kind="ExternalInput")
    out = nc.dram_tensor("out", [rows_pad, EMB], f32, kind="ExternalOutput")

    with tile.TileContext(nc) as tc:
        with tc.tile_pool(name="const", bufs=1) as cpool, \
             tc.tile_pool(name="dram", bufs=1, space="DRAM") as dpool, \
             tc.tile_pool(name="msg", bufs=msg_bufs) as mpool, \
             tc.tile_pool(name="sgen", bufs=s_bufs) as spool, \
             tc.tile_pool(name="epi", bufs=4) as epool, \
             tc.tile_pool(name="psum", bufs=psum_bufs, space="PSUM") as ppool:

            # resident tables
            idx_t = cpool.tile([P, slots], i32, name="idx_t")
            nc.sync.dma_start(idx_t[:], idx_in[:])
            rowloc_t = cpool.tile([P, slots], f32, name="rowloc_t")
            nc.sync.dma_start(rowloc_t[:], rowloc_in[:])
            val_t = cpool.tile([P, slots], f32, name="val_t")
            nc.sync.dma_start(val_t[:], val_in[:])

            # acc [p, w*EMB] holds row (w*128+p)
            acc = cpool.tile([P, w_per_core * EMB], f32, name="acc")
            nc.sync.dma_start(
                acc[:].rearrange("p (w d) -> p w d", d=EMB),
                ego_slice[:].rearrange("(w p) d -> p w d", p=P))

            # iota_f[p, t*128 + r] = r  (repeated tpw times), as f32
            # (f32 iota is exact for 0..127)
            iota_f = cpool.tile([P, tpw * P], f32, name="iota_f")
            nc.gpsimd.iota(iota_f[:].rearrange("p (t r) -> p t r", r=P),
                           pattern=[[0, tpw], [1, P]], channel_multiplier=0,
                           allow_small_or_imprecise_dtypes=True)

            # HBM buffers for inter-layer exchange
            y_hbm = [dpool.tile([rows_pad, EMB], f32, name=f"y_hbm{l}")
                     for l in range(n_layers - 1)]
            xg = [dpool.tile([n_pad, EMB], f32, name=f"xg{l}",
                             addr_space="Shared")
                  for l in range(n_layers - 1)]

            for layer in range(n_layers):
                xsrc = x0 if layer == 0 else xg[layer - 1]
                for w in range(w_per_core):
                    base = w * tpw
                    # one-hot staircase for the whole window (2 DVE ops)
                    s_all = spool.tile([P, tpw * P], f32, tag="s_all",
                                       name="s_all")
                    nc.vector.tensor_tensor(
                        out=s_all[:].rearrange("p (t r) -> p t r", r=P),
                        in0=iota_f[:].rearrange("p (t r) -> p t r", r=P),
                        in1=rowloc_t[:, base:base + tpw].to_broadcast(
                            [P, tpw, P]),
                        op=mybir.AluOpType.is_equal)
                    nc.vector.tensor_tensor(
                        out=s_all[:].rearrange("p (t r) -> p t r", r=P),
                        in0=s_all[:].rearrange("p (t r) -> p t r", r=P),
                        in1=val_t[:, base:base + tpw].to_broadcast(
                            [P, tpw, P]),
                        op=mybir.AluOpType.mult)

                    psum = ppool.tile([P, EMB], f32, name="psum")
                    for t in range(tpw):
                        j = base + t
                        g = mpool.tile([P, EMB], f32, tag="g", name="g")
                        nc.gpsimd.indirect_dma_start(
                            out=g[:], out_offset=None, in_=xsrc[:],
                            in_offset=bass.IndirectOffsetOnAxis(
                                ap=idx_t[:, j:j + 1], axis=0))
                        nc.tensor.matmul(
                            out=psum[:],
                            lhsT=s_all[:, t * P:(t + 1) * P],
                            rhs=g[:],
                            start=(t == 0), stop=(t == tpw - 1))

                    ystage = epool.tile([P, EMB], f32, tag="ystage",
                                        name="ystage")
                    nc.scalar.activation(
                        ystage[:], psum[:],
                        mybir.ActivationFunctionType.Relu)
                    nc.vector.tensor_add(
                        acc[:, w * EMB:(w + 1) * EMB],
                        acc[:, w * EMB:(w + 1) * EMB], ystage[:])
                    if layer < n_layers - 1:
                        nc.sync.dma_start(
                            y_hbm[layer][:].rearrange(
                                "(w2 p) d -> p w2 d", p=P)[:, w, :],
                            ystage[:])
                if layer < n_layers - 1:
                    nc.gpsimd.collective_compute(
                        "AllGather", mybir.AluOpType.bypass,
                        replica_groups=[list(range(N_CORES))],
                        ins=[y_hbm[layer][:]], outs=[xg[layer][:]])

            # out = acc / (n_layers + 1), scaled in place
            nc.scalar.mul(acc[:], acc[:], 1.0 / (n_layers + 1))
            nc.sync.dma_start(
                out[:].rearrange("(w p) d -> p w d", p=P),
                acc[:].rearrange("p (w d) -> p w d", d=EMB))

    nc.compile()
    return nc


def _greedy_pack(counts, tpw):
    """Pack rows (with edge `counts`) into windows of <=128 rows and
    <= tpw*128 edges.  Returns (n_windows, wmap, rlmap)."""
    cap = tpw * P
    n = len(counts)
    wmap = np.empty(n, np.int64)
    rlmap = np.empty(n, np.int64)
    w = 0
    rows_in_w = 0
    edges_in_w = 0
    for r in range(n):
        c = counts[r]
        if rows_in_w >= P or edges_in_w + c > cap:
            w += 1
            rows_in_w = 0
            edges_in_w = 0
        wmap[r] = w
        rlmap[r] = rows_in_w
        rows_in_w += 1
        edges_in_w += c
    return w + 1, wmap, rlmap


def preprocess(user_emb, item_emb, adj_val, adj_row, adj_col,
               rows_per_core=None):
    """Host-side: shard edges by dest, balance-pack dest rows into windows
    (window-space remap), build per-core index/rowloc/val tables."""
    if rows_per_core is None:
        rows_per_core = ROWS_PER_CORE
    ego = np.ascontiguousarray(
        np.concatenate([np.asarray(user_emb), np.asarray(item_emb)], axis=0),
        dtype=np.float32)
    n_nodes = ego.shape[0]
    n_cores = N_CORES

    row = np.asarray(adj_row).astype(np.int64)
    col = np.asarray(adj_col).astype(np.int64)
    val = np.asarray(adj_val).astype(np.float32)

    core_of = row // rows_per_core
    r_loc_all = row - core_of * rows_per_core

    counts_by_core = []
    for d in range(n_cores):
        counts_by_core.append(np.bincount(
            r_loc_all[core_of == d], minlength=rows_per_core).astype(np.int64))

    # pick (tpw, w_per_core) minimizing padded slot count, subject to SBUF fit
    def sbuf_kb(tpw_c, w_c):
        slots_c = w_c * tpw_c
        return (slots_c * 12 + w_c * 256 + 4 * tpw_c * 512 + 32 * 256
                + 4096) / 1024.0

    best = None
    min_tpw = max(1, int(math.ceil(max(c.max() for c in counts_by_core) / P)))
    for tpw_c in range(min_tpw, min_tpw + 40):
        packs = [_greedy_pack(c, tpw_c) for c in counts_by_core]
        w_c = max(p[0] for p in packs)
        if sbuf_kb(tpw_c, w_c) > 196.0:
            continue
        cost = w_c * tpw_c
        if best is None or cost < best[0] or (cost == best[0]
                                              and w_c < best[2]):
            best = (cost, tpw_c, w_c, packs)
    _, tpw, w_per_core, packs = best
    rows_pad = w_per_core * P
    n_pad = rows_pad * n_cores
    slots = w_per_core * tpw

    # global node -> padded position (window-space)
    pad_pos = np.empty(n_nodes, np.int64)
    for d in range(n_cores):
        _, wmap, rlmap = packs[d]
        sl = slice(d * rows_per_core, (d + 1) * rows_per_core)
        pad_pos[sl] = d * rows_pad + wmap * P + rlmap

    x0 = np.zeros((n_pad, EMB), np.float32)
    x0[pad_pos] = ego
    pcol = pad_pos[col].astype(np.int32)

    in_maps = []
    for d in range(n_cores):
        _, wmap, rlmap = packs[d]
        m = core_of == d
        r_loc = r_loc_all[m]
        w_e = wmap[r_loc]
        rl_e = rlmap[r_loc].astype(np.float32)
        order = np.argsort(w_e, kind="stable")
        w_s = w_e[order]
        rl_s = rl_e[order]
        pc_s = pcol[m][order]
        v_s = val[m][order]
        wcounts = np.bincount(w_s, minlength=w_per_core)
        idx_np = np.zeros((P, slots), np.int32)
        rowloc_np = np.zeros((P, slots), np.float32)
        val_np = np.zeros((P, slots), np.float32)
        offs = np.concatenate([[0], np.cumsum(wcounts)])[:-1]
        s_in_w = np.arange(len(w_s)) - offs[w_s]
        t = s_in_w >> 7
        p = s_in_w & 127
        j = w_s * tpw + t
        idx_np[p, j] = pc_s
        rowloc_np[p, j] = rl_s
        val_np[p, j] = v_s
        in_maps.append({
            "x0": x0,
            "ego_slice": np.ascontiguousarray(
                x0.reshape(n_cores, rows_pad, EMB)[d]),
            "idx": idx_np,
            "rowloc": rowloc_np,
            "val": val_np,
        })
    return tpw, w_per_core, in_maps, pad_pos


def assemble_output(results, pad_pos, rows_pad):
    """Map per-core window-space outputs back to node order."""
    big = np.concatenate([results[d]["out"] for d in range(N_CORES)], axis=0)
    full = big[pad_pos]
    return full[:N_USERS], full[N_USERS:]


def kernel(user_emb, item_emb, adj_val, adj_row, adj_col):
    from concourse.bass_utils import run_bass_kernel_spmd

    tpw, w_per_core, in_maps, pad_pos = preprocess(
        user_emb, item_emb, adj_val, adj_row, adj_col)

    key = (tpw, w_per_core)
    if key not in _COMPILED:
        _COMPILED[key] = build_program(tpw, w_per_core=w_per_core)
    nc = _COMPILED[key]

    res = run_bass_kernel_spmd(nc, in_maps, core_ids=list(range(N_CORES)))
    return assemble_output(res.results, pad_pos, w_per_core * P)



# revision 13
# speedup vs baseline: 1.2111x; 1.1441x over previous
"""GCCF encoder (3-layer LightGCN-style propagation) on 8 TRN2 NeuronCores.

Strategy (dest-partitioned SpMM):
  - Node dim sharded 8 ways: core d owns dest rows [d*18750, (d+1)*18750).
  - Core-local edges sorted by dest row, grouped into 148 windows of 128
    dest rows, padded to a fixed TPW tiles of 128 edges per window.
  - Per edge-tile: indirect-DMA gather of the 128 source rows from an
    HBM-resident full embedding table; a one-hot "staircase" matrix S
    (S[e, r] = val_e * (rowloc_e == r)) generated on-chip by the vector
    engine; PE matmul  psum[128 rows, 64] += S.T @ msgs  accumulates the
    segment sum for the window.
  - Window epilogue: relu (scalar engine) -> accumulate into SBUF-resident
    acc -> write y to HBM; AllGather (8 cores) rebuilds the full table for
    the next layer.  acc/4 is the output.
All data-dependent structure lives in input tensors (gather indices,
rowloc, val); the Bass program itself is identical on all 8 cores.
"""

import math
import numpy as np

N_USERS = 100000
N_ITEMS = 50000
N_NODES = N_USERS + N_ITEMS
EMB = 64
N_LAYERS = 3
N_CORES = 8
P = 128

ROWS_PER_CORE = N_NODES // N_CORES          # 18750
W_PER_CORE = math.ceil(ROWS_PER_CORE / P)   # 147 -> pad rows to 148*128
ROWS_PAD = W_PER_CORE * P                   # 18816? (147*128=18816)
N_PAD = ROWS_PAD * N_CORES

_COMPILED = {}


def build_program(tpw, n_layers=N_LAYERS, w_per_core=None, rows_pad=None,
                  n_pad=None, msg_bufs=32, s_bufs=8, psum_bufs=8):
    """Build + compile the SPMD Bass program. Returns (nc, names)."""
    import concourse.bass as bass
    from concourse import bacc, tile, mybir

    if w_per_core is None:
        w_per_core = W_PER_CORE
    if rows_pad is None:
        rows_pad = w_per_core * P
    if n_pad is None:
        n_pad = rows_pad * N_CORES
    slots = w_per_core * tpw

    f32 = mybir.dt.float32
    i32 = mybir.dt.int32

    nc = bacc.Bacc("TRN2", target_bir_lowering=False, debug=False,
                   enable_asserts=False, num_devices=N_CORES)

    x0 = nc.dram_tensor("x0", [n_pad, EMB], f32, kind="ExternalInput")
    ego_slice = nc.dram_tensor("ego_slice", [rows_pad, EMB], f32,
                               kind="ExternalInput")
    idx_in = nc.dram_tensor("idx", [P, slots], i32, kind="ExternalInput")
    rowloc_in = nc.dram_tensor("rowloc", [P, slots], f32, kind="ExternalInput")
    val_in = nc.dram_tensor("val", [P, slots], f32, kind="ExternalInput")
    out = nc.dram_tensor("out", [rows_pad, EMB], f32, kind="ExternalOutput")

    with tile.TileContext(nc) as tc:
        with tc.tile_pool(name="const", bufs=1) as cpool, \
             tc.tile_pool(name="dram", bufs=1, space="DRAM") as dpool, \
             tc.tile_pool(name="msg", bufs=msg_bufs) as mpool, \
             tc.tile_pool(name="sgen", bufs=s_bufs) as spool, \
             tc.tile_pool(name="epi", bufs=4) as epool, \
             tc.tile_pool(name="psum", bufs=psum_bufs, space="PSUM") as ppool:

            # resident tables
            idx_t = cpool.tile([P, slots], i32, name="idx_t")
            nc.sync.dma_start(idx_t[:], idx_in[:])
            rowloc_t = cpool.tile([P, slots], f32, name="rowloc_t")
            nc.sync.dma_start(rowloc_t[:], rowloc_in[:])
            val_t = cpool.tile([P, slots], f32, name="val_t")
            nc.sync.dma_start(val_t[:], val_in[:])

            # acc [p, w*EMB] holds row (w*128+p)
            acc = cpool.tile([P, w_per_core * EMB], f32, name="acc")
            nc.sync.dma_start(
                acc[:].rearrange("p (w d) -> p w d", d=EMB),
                ego_slice[:].rearrange("(w p) d -> p w d", p=P))

            # iota_f[p, r] = r, as f32 (exact for 0..127)
            iota_f = cpool.tile([P, P], f32, name="iota_f")
            nc.gpsimd.iota(iota_f[:], pattern=[[1, P]], channel_multiplier=0,
                           allow_small_or_imprecise_dtypes=True)

            # HBM buffers for inter-layer exchange
            y_hbm = [dpool.tile([rows_pad, EMB], f32, name=f"y_hbm{l}")
                     for l in range(n_layers - 1)]
            xg = [dpool.tile([n_pad, EMB], f32, name=f"xg{l}",
                             addr_space="Shared")
                  for l in range(n_layers - 1)]

            for layer in range(n_layers):
                xsrc = x0 if layer == 0 else xg[layer - 1]
                for w in range(w_per_core):
                    base = w * tpw
                    psum = ppool.tile([P, EMB], f32, name="psum")
                    for t in range(tpw):
                        j = base + t
                        g = mpool.tile([P, EMB], f32, tag="g", name="g")
                        nc.gpsimd.indirect_dma_start(
                            out=g[:], out_offset=None, in_=xsrc[:],
                            in_offset=bass.IndirectOffsetOnAxis(
                                ap=idx_t[:, j:j + 1], axis=0))
                        # one-hot staircase: S[e, r] = val_e * (r == rowloc_e)
                        s_t = spool.tile([P, P], f32, tag="s_t", name="s_t")
                        nc.vector.tensor_scalar(
                            out=s_t[:], in0=iota_f[:],
                            scalar1=rowloc_t[:, j:j + 1],
                            scalar2=val_t[:, j:j + 1],
                            op0=mybir.AluOpType.is_equal,
                            op1=mybir.AluOpType.mult)
                        nc.tensor.matmul(
                            out=psum[:],
                            lhsT=s_t[:],
                            rhs=g[:],
                            start=(t == 0), stop=(t == tpw - 1))

                    ystage = epool.tile([P, EMB], f32, tag="ystage",
                                        name="ystage")
                    nc.scalar.activation(
                        ystage[:], psum[:],
                        mybir.ActivationFunctionType.Relu)
                    nc.vector.tensor_add(
                        acc[:, w * EMB:(w + 1) * EMB],
                        acc[:, w * EMB:(w + 1) * EMB], ystage[:])
                    if layer < n_layers - 1:
                        nc.sync.dma_start(
                            y_hbm[layer][:].rearrange(
                                "(w2 p) d -> p w2 d", p=P)[:, w, :],
                            ystage[:])
                if layer < n_layers - 1:
                    nc.gpsimd.collective_compute(
                        "AllGather", mybir.AluOpType.bypass,
                        replica_groups=[list(range(N_CORES))],
                        ins=[y_hbm[layer][:]], outs=[xg[layer][:]])

            # out = acc / (n_layers + 1), scaled in place
            nc.scalar.mul(acc[:], acc[:], 1.0 / (n_layers + 1))
            nc.sync.dma_start(
                out[:].rearrange("(w p) d -> p w d", p=P),
                acc[:].rearrange("p (w d) -> p w d", d=EMB))

    nc.compile()
    return nc


def _greedy_pack(counts, tpw):
    """Pack rows (with edge `counts`) into windows of <=128 rows and
    <= tpw*128 edges.  Returns (n_windows, wmap, rlmap)."""
    cap = tpw * P
    n = len(counts)
    wmap = np.empty(n, np.int64)
    rlmap = np.empty(n, np.int64)
    w = 0
    rows_in_w = 0
    edges_in_w = 0
    for r in range(n):
        c = counts[r]
        if rows_in_w >= P or edges_in_w + c > cap:
            w += 1
            rows_in_w = 0
            edges_in_w = 0
        wmap[r] = w
        rlmap[r] = rows_in_w
        rows_in_w += 1
        edges_in_w += c
    return w + 1, wmap, rlmap


def preprocess(user_emb, item_emb, adj_val, adj_row, adj_col,
               rows_per_core=None):
    """Host-side: shard edges by dest, balance-pack dest rows into windows
    (window-space remap), build per-core index/rowloc/val tables."""
    if rows_per_core is None:
        rows_per_core = ROWS_PER_CORE
    ego = np.ascontiguousarray(
        np.concatenate([np.asarray(user_emb), np.asarray(item_emb)], axis=0),
        dtype=np.float32)
    n_nodes = ego.shape[0]
    n_cores = N_CORES

    row = np.asarray(adj_row).astype(np.int64)
    col = np.asarray(adj_col).astype(np.int64)
    val = np.asarray(adj_val).astype(np.float32)

    core_of = row // rows_per_core
    r_loc_all = row - core_of * rows_per_core

    counts_by_core = []
    for d in range(n_cores):
        counts_by_core.append(np.bincount(
            r_loc_all[core_of == d], minlength=rows_per_core).astype(np.int64))

    # pick (tpw, w_per_core) minimizing padded slot count, subject to SBUF fit
    def sbuf_kb(tpw_c, w_c):
        slots_c = w_c * tpw_c
        return (slots_c * 12 + w_c * 256 + 4 * tpw_c * 512 + 32 * 256
                + 4096) / 1024.0

    best = None
    min_tpw = max(1, int(math.ceil(max(c.max() for c in counts_by_core) / P)))
    for tpw_c in range(min_tpw, min_tpw + 40):
        packs = [_greedy_pack(c, tpw_c) for c in counts_by_core]
        w_c = max(p[0] for p in packs)
        if sbuf_kb(tpw_c, w_c) > 196.0:
            continue
        cost = w_c * tpw_c
        if best is None or cost < best[0] or (cost == best[0]
                                              and w_c < best[2]):
            best = (cost, tpw_c, w_c, packs)
    _, tpw, w_per_core, packs = best
    rows_pad = w_per_core * P
    n_pad = rows_pad * n_cores
    slots = w_per_core * tpw

    # global node -> padded position (window-space)
    pad_pos = np.empty(n_nodes, np.int64)
    for d in range(n_cores):
        _, wmap, rlmap = packs[d]
        sl = slice(d * rows_per_core, (d + 1) * rows_per_core)
        pad_pos[sl] = d * rows_pad + wmap * P + rlmap

    x0 = np.zeros((n_pad, EMB), np.float32)
    x0[pad_pos] = ego
    pcol = pad_pos[col].astype(np.int32)

    in_maps = []
    for d in range(n_cores):
        _, wmap, rlmap = packs[d]
        m = core_of == d
        r_loc = r_loc_all[m]
        w_e = wmap[r_loc]
        rl_e = rlmap[r_loc].astype(np.float32)
        order = np.argsort(w_e, kind="stable")
        w_s = w_e[order]
        rl_s = rl_e[order]
        pc_s = pcol[m][order]
        v_s = val[m][order]
        wcounts = np.bincount(w_s, minlength=w_per_core)
        idx_np = np.zeros((P, slots), np.int32)
        rowloc_np = np.zeros((P, slots), np.float32)
        val_np = np.zeros((P, slots), np.float32)
        offs = np.concatenate([[0], np.cumsum(wcounts)])[:-1]
        s_in_w = np.arange(len(w_s)) - offs[w_s]
        t = s_in_w >> 7
        p = s_in_w & 127
        j = w_s * tpw + t
        idx_np[p, j] = pc_s
        rowloc_np[p, j] = rl_s
        val_np[p, j] = v_s
        in_maps.append({
            "x0": x0,
            "ego_slice": np.ascontiguousarray(
                x0.reshape(n_cores, rows_pad, EMB)[d]),
            "idx": idx_np,
            "rowloc": rowloc_np,
            "val": val_np,
        })
    return tpw, w_per_core, in_maps, pad_pos


def assemble_output(results, pad_pos, rows_pad):
    """Map per-core window-space outputs back to node order."""
    big = np.concatenate([results[d]["out"] for d in range(N_CORES)], axis=0)
    full = big[pad_pos]
    return full[:N_USERS], full[N_USERS:]


def kernel(user_emb, item_emb, adj_val, adj_row, adj_col):
    from concourse.bass_utils import run_bass_kernel_spmd

    tpw, w_per_core, in_maps, pad_pos = preprocess(
        user_emb, item_emb, adj_val, adj_row, adj_col)

    key = (tpw, w_per_core)
    if key not in _COMPILED:
        _COMPILED[key] = build_program(tpw, w_per_core=w_per_core)
    nc = _COMPILED[key]

    res = run_bass_kernel_spmd(nc, in_maps, core_ids=list(range(N_CORES)))
    return assemble_output(res.results, pad_pos, w_per_core * P)



# revision 15
# speedup vs baseline: 1.2116x; 1.0004x over previous
"""GCCF encoder (3-layer LightGCN-style propagation) on 8 TRN2 NeuronCores.

Strategy (dest-partitioned SpMM):
  - Node dim sharded 8 ways: core d owns dest rows [d*18750, (d+1)*18750).
  - Core-local edges sorted by dest row, grouped into 148 windows of 128
    dest rows, padded to a fixed TPW tiles of 128 edges per window.
  - Per edge-tile: indirect-DMA gather of the 128 source rows from an
    HBM-resident full embedding table; a one-hot "staircase" matrix S
    (S[e, r] = val_e * (rowloc_e == r)) generated on-chip by the vector
    engine; PE matmul  psum[128 rows, 64] += S.T @ msgs  accumulates the
    segment sum for the window.
  - Window epilogue: relu (scalar engine) -> accumulate into SBUF-resident
    acc -> write y to HBM; AllGather (8 cores) rebuilds the full table for
    the next layer.  acc/4 is the output.
All data-dependent structure lives in input tensors (gather indices,
rowloc, val); the Bass program itself is identical on all 8 cores.
"""

import math
import numpy as np

N_USERS = 100000
N_ITEMS = 50000
N_NODES = N_USERS + N_ITEMS
EMB = 64
N_LAYERS = 3
N_CORES = 8
P = 128

ROWS_PER_CORE = N_NODES // N_CORES          # 18750
W_PER_CORE = math.ceil(ROWS_PER_CORE / P)   # 147 -> pad rows to 148*128
ROWS_PAD = W_PER_CORE * P                   # 18816? (147*128=18816)
N_PAD = ROWS_PAD * N_CORES

_COMPILED = {}


def build_program(tpw, n_layers=N_LAYERS, w_per_core=None, rows_pad=None,
                  n_pad=None, msg_bufs=32, s_bufs=8, psum_bufs=8):
    """Build + compile the SPMD Bass program. Returns (nc, names)."""
    import concourse.bass as bass
    from concourse import bacc, tile, mybir

    if w_per_core is None:
        w_per_core = W_PER_CORE
    if rows_pad is None:
        rows_pad = w_per_core * P
    if n_pad is None:
        n_pad = rows_pad * N_CORES
    slots = w_per_core * tpw

    f32 = mybir.dt.float32
    i32 = mybir.dt.int32

    nc = bacc.Bacc("TRN2", target_bir_lowering=False, debug=False,
                   enable_asserts=False, num_devices=N_CORES)

    x0 = nc.dram_tensor("x0", [n_pad, EMB], f32, kind="ExternalInput")
    ego_slice = nc.dram_tensor("ego_slice", [rows_pad, EMB], f32,
                               kind="ExternalInput")
    idx_in = nc.dram_tensor("idx", [P, slots], i32, kind="ExternalInput")
    rowloc_in = nc.dram_tensor("rowloc", [P, slots], f32, kind="ExternalInput")
    val_in = nc.dram_tensor("val", [P, slots], f32, kind="ExternalInput")
    out = nc.dram_tensor("out", [rows_pad, EMB], f32, kind="ExternalOutput")

    with tile.TileContext(nc) as tc:
        with tc.tile_pool(name="const", bufs=1) as cpool, \
             tc.tile_pool(name="dram", bufs=1, space="DRAM") as dpool, \
             tc.tile_pool(name="msg", bufs=msg_bufs) as mpool, \
             tc.tile_pool(name="sgen", bufs=s_bufs) as spool, \
             tc.tile_pool(name="epi", bufs=4) as epool, \
             tc.tile_pool(name="psum", bufs=psum_bufs, space="PSUM") as ppool:

            # resident tables
            idx_t = cpool.tile([P, slots], i32, name="idx_t")
            nc.sync.dma_start(idx_t[:], idx_in[:])
            rowloc_t = cpool.tile([P, slots], f32, name="rowloc_t")
            nc.sync.dma_start(rowloc_t[:], rowloc_in[:])
            val_t = cpool.tile([P, slots], f32, name="val_t")
            nc.sync.dma_start(val_t[:], val_in[:])

            # acc [p, w*EMB] holds row (w*128+p)
            acc = cpool.tile([P, w_per_core * EMB], f32, name="acc")
            nc.sync.dma_start(
                acc[:].rearrange("p (w d) -> p w d", d=EMB),
                ego_slice[:].rearrange("(w p) d -> p w d", p=P))

            # iota_f[p, r] = r, as f32 (exact for 0..127)
            iota_f = cpool.tile([P, P], f32, name="iota_f")
            nc.gpsimd.iota(iota_f[:], pattern=[[1, P]], channel_multiplier=0,
                           allow_small_or_imprecise_dtypes=True)
            # negated tables for the ACT-based one-hot construction
            neg_rowloc_t = cpool.tile([P, slots], f32, name="neg_rowloc_t")
            nc.vector.tensor_scalar_mul(neg_rowloc_t[:], rowloc_t[:], -1.0)
            neg_val_t = cpool.tile([P, slots], f32, name="neg_val_t")
            nc.vector.tensor_scalar_mul(neg_val_t[:], val_t[:], -1.0)

            # HBM buffers for inter-layer exchange
            y_hbm = [dpool.tile([rows_pad, EMB], f32, name=f"y_hbm{l}")
                     for l in range(n_layers - 1)]
            xg = [dpool.tile([n_pad, EMB], f32, name=f"xg{l}",
                             addr_space="Shared")
                  for l in range(n_layers - 1)]

            for layer in range(n_layers):
                xsrc = x0 if layer == 0 else xg[layer - 1]
                for w in range(w_per_core):
                    base = w * tpw
                    psum = ppool.tile([P, EMB], f32, name="psum")
                    for t in range(tpw):
                        j = base + t
                        g = mpool.tile([P, EMB], f32, tag="g", name="g")
                        nc.gpsimd.indirect_dma_start(
                            out=g[:], out_offset=None, in_=xsrc[:],
                            in_offset=bass.IndirectOffsetOnAxis(
                                ap=idx_t[:, j:j + 1], axis=0))
                        # one-hot staircase: S[e, r] = val_e * (r == rowloc_e)
                        # built on the (otherwise idle) scalar engine:
                        #   a = (r - rowloc)^2 ;  S = relu(val - val*a)
                        a_t = spool.tile([P, P], f32, tag="a_t", name="a_t")
                        nc.scalar.activation(
                            a_t[:], iota_f[:],
                            mybir.ActivationFunctionType.Square,
                            bias=neg_rowloc_t[:, j:j + 1])
                        s_t = spool.tile([P, P], f32, tag="s_t", name="s_t")
                        nc.scalar.activation(
                            s_t[:], a_t[:],
                            mybir.ActivationFunctionType.Relu,
                            bias=val_t[:, j:j + 1],
                            scale=neg_val_t[:, j:j + 1])
                        nc.tensor.matmul(
                            out=psum[:],
                            lhsT=s_t[:],
                            rhs=g[:],
                            start=(t == 0), stop=(t == tpw - 1))

                    ystage = epool.tile([P, EMB], f32, tag="ystage",
                                        name="ystage")
                    nc.scalar.activation(
                        ystage[:], psum[:],
                        mybir.ActivationFunctionType.Relu)
                    nc.vector.tensor_add(
                        acc[:, w * EMB:(w + 1) * EMB],
                        acc[:, w * EMB:(w + 1) * EMB], ystage[:])
                    if layer < n_layers - 1:
                        nc.sync.dma_start(
                            y_hbm[layer][:].rearrange(
                                "(w2 p) d -> p w2 d", p=P)[:, w, :],
                            ystage[:])
                if layer < n_layers - 1:
                    nc.gpsimd.collective_compute(
                        "AllGather", mybir.AluOpType.bypass,
                        replica_groups=[list(range(N_CORES))],
                        ins=[y_hbm[layer][:]], outs=[xg[layer][:]])

            # out = acc / (n_layers + 1), scaled in place
            nc.scalar.mul(acc[:], acc[:], 1.0 / (n_layers + 1))
            nc.sync.dma_start(
                out[:].rearrange("(w p) d -> p w d", p=P),
                acc[:].rearrange("p (w d) -> p w d", d=EMB))

    nc.compile()
    return nc


def _greedy_pack(counts, tpw):
    """Pack rows (with edge `counts`) into windows of <=128 rows and
    <= tpw*128 edges.  Returns (n_windows, wmap, rlmap)."""
    cap = tpw * P
    n = len(counts)
    wmap = np.empty(n, np.int64)
    rlmap = np.empty(n, np.int64)
    w = 0
    rows_in_w = 0
    edges_in_w = 0
    for r in range(n):
        c = counts[r]
        if rows_in_w >= P or edges_in_w + c > cap:
            w += 1
            rows_in_w = 0
            edges_in_w = 0
        wmap[r] = w
        rlmap[r] = rows_in_w
        rows_in_w += 1
        edges_in_w += c
    return w + 1, wmap, rlmap


def preprocess(user_emb, item_emb, adj_val, adj_row, adj_col,
               rows_per_core=None):
    """Host-side: shard edges by dest, balance-pack dest rows into windows
    (window-space remap), build per-core index/rowloc/val tables."""
    if rows_per_core is None:
        rows_per_core = ROWS_PER_CORE
    ego = np.ascontiguousarray(
        np.concatenate([np.asarray(user_emb), np.asarray(item_emb)], axis=0),
        dtype=np.float32)
    n_nodes = ego.shape[0]
    n_cores = N_CORES

    row = np.asarray(adj_row).astype(np.int64)
    col = np.asarray(adj_col).astype(np.int64)
    val = np.asarray(adj_val).astype(np.float32)

    core_of = row // rows_per_core
    r_loc_all = row - core_of * rows_per_core

    counts_by_core = []
    for d in range(n_cores):
        counts_by_core.append(np.bincount(
            r_loc_all[core_of == d], minlength=rows_per_core).astype(np.int64))

    # pick (tpw, w_per_core) minimizing padded slot count, subject to SBUF fit
    def sbuf_kb(tpw_c, w_c):
        slots_c = w_c * tpw_c
        return (slots_c * 12 + w_c * 256 + 4 * tpw_c * 512 + 32 * 256
                + 4096) / 1024.0

    best = None
    min_tpw = max(1, int(math.ceil(max(c.max() for c in counts_by_core) / P)))
    for tpw_c in range(min_tpw, min_tpw + 40):
        packs = [_greedy_pack(c, tpw_c) for c in counts_by_core]
        w_c = max(p[0] for p in packs)
        if sbuf_kb(tpw_c, w_c) > 196.0:
            continue
        cost = w_c * tpw_c
        if best is None or cost < best[0] or (cost == best[0]
                                              and w_c < best[2]):
            best = (cost, tpw_c, w_c, packs)
    _, tpw, w_per_core, packs = best
    rows_pad = w_per_core * P
    n_pad = rows_pad * n_cores
    slots = w_per_core * tpw

    # global node -> padded position (window-space)
    pad_pos = np.empty(n_nodes, np.int64)
    for d in range(n_cores):
        _, wmap, rlmap = packs[d]
        sl = slice(d * rows_per_core, (d + 1) * rows_per_core)
        pad_pos[sl] = d * rows_pad + wmap * P + rlmap

    x0 = np.zeros((n_pad, EMB), np.float32)
    x0[pad_pos] = ego
    pcol = pad_pos[col].astype(np.int32)

    in_maps = []
    for d in range(n_cores):
        _, wmap, rlmap = packs[d]
        m = core_of == d
        r_loc = r_loc_all[m]
        w_e = wmap[r_loc]
        rl_e = rlmap[r_loc].astype(np.float32)
        order = np.argsort(w_e, kind="stable")
        w_s = w_e[order]
        rl_s = rl_e[order]
        pc_s = pcol[m][order]
        v_s = val[m][order]
        wcounts = np.bincount(w_s, minlength=w_per_core)
        idx_np = np.zeros((P, slots), np.int32)
        rowloc_np = np.zeros((P, slots), np.float32)
        val_np = np.zeros((P, slots), np.float32)
        offs = np.concatenate([[0], np.cumsum(wcounts)])[:-1]
        s_in_w = np.arange(len(w_s)) - offs[w_s]
        t = s_in_w >> 7
        p = s_in_w & 127
        j = w_s * tpw + t
        idx_np[p, j] = pc_s
        rowloc_np[p, j] = rl_s
        val_np[p, j] = v_s
        in_maps.append({
            "x0": x0,
            "ego_slice": np.ascontiguousarray(
                x0.reshape(n_cores, rows_pad, EMB)[d]),
            "idx": idx_np,
            "rowloc": rowloc_np,
            "val": val_np,
        })
    return tpw, w_per_core, in_maps, pad_pos


def assemble_output(results, pad_pos, rows_pad):
    """Map per-core window-space outputs back to node order."""
    big = np.concatenate([results[d]["out"] for d in range(N_CORES)], axis=0)
    full = big[pad_pos]
    return full[:N_USERS], full[N_USERS:]


def kernel(user_emb, item_emb, adj_val, adj_row, adj_col):
    from concourse.bass_utils import run_bass_kernel_spmd

    tpw, w_per_core, in_maps, pad_pos = preprocess(
        user_emb, item_emb, adj_val, adj_row, adj_col)

    key = (tpw, w_per_core)
    if key not in _COMPILED:
        _COMPILED[key] = build_program(tpw, w_per_core=w_per_core)
    nc = _COMPILED[key]

    res = run_bass_kernel_spmd(nc, in_maps, core_ids=list(range(N_CORES)))
    return assemble_output(res.results, pad_pos, w_per_core * P)

